# revision 1
# baseline (speedup 1.0000x reference)
"""Bass/Trainium2 kernel for nn_CustomLoss_87952340287807.

Loss over B=8,388,608 Euler-angle triples:
    per-sample = 1 - |cross(vo, vt)| + dot(vo, vt),  summed.
vo/vt are unit vectors, so |cross| = sqrt(1 - dot^2) and only dot is needed.

dot = cosD*(u*U + v*V) + sinD*(u*V - v*U) + w*W
  u = sin(p~)cos(r~), v = sin(r~), w = cos(p~)cos(r~)   (o side; caps = t side)
  D = 2*pi*(yt - yo)
All trig from the Sin LUT (valid domain [-pi, pi]):
  sin(2pi(x-.5)) = Sin(2pi*x - pi)
  cos(2pi(x-.5)) = 1 - 2*h^2,  h = Sin(pi*x - pi/2)
  sinD = hD*(2 - 4*jD^2), cosD = 1 - 2*hD^2,  hD = Sin(pi*d), jD = Sin(pi*d/2)

Sharding: pure data-parallel, batch split across 8 NeuronCores; each core
returns per-partition partial sums of dot and cross-norm; host reduces.
"""
import sys

import numpy as np

if "/opt/trn_rl_repo" not in sys.path:
    sys.path.insert(0, "/opt/trn_rl_repo")

import concourse.bacc as bacc
import concourse.mybir as mybir
import concourse.tile as tile
from concourse.bass_utils import run_bass_kernel_spmd

B = 8388608
NCORES = 8
S = B // NCORES          # 1,048,576 samples per core
P = 128
F = 2048                 # samples per partition per tile
NT = S // (P * F)        # 4 tiles

AF = mybir.ActivationFunctionType
ALU = mybir.AluOpType
dt = mybir.dt
f32, f16 = dt.float32, dt.float16
PI = float(np.pi)

_cache = {}
last_results = None


def _build():
    nc = bacc.Bacc("TRN2", target_bir_lowering=False, debug=False)
    o_in = nc.declare_dram_parameter("out_in", [S, 3], f32, isOutput=False)
    t_in = nc.declare_dram_parameter("tgt_in", [S, 3], f32, isOutput=False)
    res = nc.declare_dram_parameter("res", [P, 2], f32, isOutput=True)

    o_flat = o_in.ap().rearrange("(p n) c -> p (n c)", p=P)
    t_flat = t_in.ap().rearrange("(p n) c -> p (n c)", p=P)

    with tile.TileContext(nc) as tc:
        with tc.tile_pool(name="consts", bufs=1) as cpool, \
             tc.tile_pool(name="raw", bufs=2) as rawpool, \
             tc.tile_pool(name="sb", bufs=1) as pool, \
             tc.tile_pool(name="persist", bufs=1) as ppool:
            consts = {}
            for i, val in enumerate([-PI, -PI / 2, 1.0]):
                ct = cpool.tile([P, 1], f32, name=f"cst{i}", tag=f"cst{i}")
                nc.vector.memset(ct[:], val)
                consts[val] = ct[:]

            d2_all = ppool.tile([P, NT * F], f16, name="d2_all", tag="d2_all")
            dacc = ppool.tile([P, NT], f32, name="dacc", tag="dacc")
            cacc = ppool.tile([P, NT], f32, name="cacc", tag="cacc")

            def mk(tag, cols=F, d=f16):
                return pool.tile([P, cols], d, name=tag, tag=tag)

            for i in range(NT):
                raw_o = rawpool.tile([P, 3 * F], f16, name="raw_o", tag="raw_o")
                nc.gpsimd.dma_start(raw_o[:], o_flat[:, i * 3 * F:(i + 1) * 3 * F])
                raw_t = rawpool.tile([P, 3 * F], f16, name="raw_t", tag="raw_t")
                nc.gpsimd.dma_start(raw_t[:], t_flat[:, i * 3 * F:(i + 1) * 3 * F])

                ov = raw_o[:].rearrange("p (n c) -> p c n", c=3)
                tv = raw_t[:].rearrange("p (n c) -> p c n", c=3)
                yo, yt = ov[:, 0, :], tv[:, 0, :]
                pr_o, pr_t = ov[:, 1:3, :], tv[:, 1:3, :]

                # full-angle sines [sp | sr] and half-angle sines [hp | hr]
                sc_o = mk("sc_o", 2 * F)
                nc.scalar.activation(sc_o[:].rearrange("p (c n) -> p c n", c=2),
                                     pr_o, AF.Sin, bias=consts[-PI], scale=2 * PI)
                sc_t = mk("sc_t", 2 * F)
                nc.scalar.activation(sc_t[:].rearrange("p (c n) -> p c n", c=2),
                                     pr_t, AF.Sin, bias=consts[-PI], scale=2 * PI)
                hh_o = mk("hh_o", 2 * F)
                nc.scalar.activation(hh_o[:].rearrange("p (c n) -> p c n", c=2),
                                     pr_o, AF.Sin, bias=consts[-PI / 2], scale=PI)
                hh_t = mk("hh_t", 2 * F)
                nc.scalar.activation(hh_t[:].rearrange("p (c n) -> p c n", c=2),
                                     pr_t, AF.Sin, bias=consts[-PI / 2], scale=PI)

                sp, sr = sc_o[:, :F], sc_o[:, F:]
                SP, SR = sc_t[:, :F], sc_t[:, F:]

                # cos = 1 - 2h^2 (paired [cp | cr])
                qq = mk("qq", 2 * F)
                nc.vector.tensor_mul(qq[:], hh_o[:], hh_o[:])
                c_o = mk("c_o", 2 * F)
                nc.vector.tensor_scalar(c_o[:], qq[:], -2.0, 1.0, ALU.mult, ALU.add)
                qq2 = mk("qq", 2 * F)
                nc.vector.tensor_mul(qq2[:], hh_t[:], hh_t[:])
                c_t = mk("c_t", 2 * F)
                nc.vector.tensor_scalar(c_t[:], qq2[:], -2.0, 1.0, ALU.mult, ALU.add)
                cp, cr = c_o[:, :F], c_o[:, F:]
                CP, CR = c_t[:, :F], c_t[:, F:]

                # delta chain: d = yt - yo
                dlt = mk("dlt")
                nc.vector.tensor_sub(dlt[:], yt, yo)
                hj = mk("hj", 2 * F)
                nc.scalar.activation(hj[:, :F], dlt[:], AF.Sin, scale=PI)
                nc.scalar.activation(hj[:, F:], dlt[:], AF.Sin, scale=PI / 2)
                hD, jD = hj[:, :F], hj[:, F:]
                t1 = mk("t1")
                nc.vector.tensor_mul(t1[:], hD, hD)
                cD = mk("cD")
                nc.vector.tensor_scalar(cD[:], t1[:], -2.0, 1.0, ALU.mult, ALU.add)
                t1b = mk("t1")
                nc.vector.tensor_mul(t1b[:], jD, jD)
                t2 = mk("t2")
                nc.vector.tensor_scalar(t2[:], t1b[:], -4.0, 2.0, ALU.mult, ALU.add)
                sD = mk("sD")
                nc.vector.tensor_mul(sD[:], hD, t2[:])

                # bilinear chain
                u = mk("u")
                nc.vector.tensor_mul(u[:], sp, cr)
                w = mk("w")
                nc.vector.tensor_mul(w[:], cp, cr)
                U_ = mk("U_")
                nc.vector.tensor_mul(U_[:], SP, CR)
                W_ = mk("W_")
                nc.vector.tensor_mul(W_[:], CP, CR)
                m1 = mk("t1")
                nc.vector.tensor_mul(m1[:], u[:], U_[:])
                m2 = mk("t2")
                nc.vector.tensor_mul(m2[:], sr, SR)
                a = mk("t3")
                nc.vector.tensor_add(a[:], m1[:], m2[:])
                m3 = mk("t1")
                nc.vector.tensor_mul(m3[:], u[:], SR)
                m4 = mk("t2")
                nc.vector.tensor_mul(m4[:], sr, U_[:])
                b = mk("t4")
                nc.vector.tensor_sub(b[:], m3[:], m4[:])
                g = mk("t1")
                nc.vector.tensor_mul(g[:], w[:], W_[:])
                p1 = mk("t2")
                nc.vector.tensor_mul(p1[:], cD[:], a[:])
                q1 = mk("t3")
                nc.vector.tensor_mul(q1[:], sD[:], b[:])
                r1 = mk("t4")
                nc.vector.tensor_add(r1[:], p1[:], q1[:])
                dot = mk("dot")
                nc.vector.tensor_add(dot[:], r1[:], g[:])

                # sum(dot) -> dacc[:, i];  d2 = min(dot^2, 1) -> d2_all
                scr = mk("t1")
                nc.vector.tensor_scalar(scr[:], dot[:], 1.0, 0.0, ALU.mult, ALU.add,
                                        accum_out=dacc[:, i:i + 1])
                d2 = mk("t2")
                nc.vector.tensor_mul(d2[:], dot[:], dot[:])
                nc.vector.tensor_scalar_min(d2_all[:, i * F:(i + 1) * F], d2[:], 1.0)

            # phase B: sqrt in NT chunks; chunk 0 is gated on the LAST tile's
            # d2 (via a bias tile equal to 1.0 computed from it) so the act
            # table switches exactly once; later chunks chain via WAW on the
            # shared cn scratch.
            gate = ppool.tile([P, 1], f32, name="gate", tag="gate")
            nc.vector.tensor_scalar(gate[:], d2_all[:, NT * F - 1:NT * F],
                                    0.0, 1.0, ALU.mult, ALU.add)
            for k in range(NT):
                cn = pool.tile([P, F], f16, name="cn", tag="cn")
                bias = gate[:] if k == 0 else consts[1.0]
                nc.scalar.activation(cn[:], d2_all[:, k * F:(k + 1) * F], AF.Sqrt,
                                     bias=bias, scale=-1.0,
                                     accum_out=cacc[:, k:k + 1])

            both = ppool.tile([P, 2], f32, name="both", tag="both")
            nc.vector.tensor_reduce(both[:, 0:1], dacc[:], mybir.AxisListType.X,
                                    ALU.add)
            nc.vector.tensor_reduce(both[:, 1:2], cacc[:], mybir.AxisListType.X,
                                    ALU.add)
            nc.sync.dma_start(res[:], both[:])

    nc.compile()
    return nc


def kernel(output: np.ndarray, target: np.ndarray) -> np.ndarray:
    global last_results
    if "nc" not in _cache:
        _cache["nc"] = _build()
    nc = _cache["nc"]

    output = np.ascontiguousarray(output, dtype=np.float32)
    target = np.ascontiguousarray(target, dtype=np.float32)
    in_maps = [
        {"out_in": output[c * S:(c + 1) * S], "tgt_in": target[c * S:(c + 1) * S]}
        for c in range(NCORES)
    ]
    r = run_bass_kernel_spmd(nc, in_maps, list(range(NCORES)))
    last_results = r

    total = np.float64(B)
    for c in range(NCORES):
        out = r.results[c]["res"].astype(np.float64)
        total += out[:, 0].sum() - out[:, 1].sum()
    return np.float32(total)

